# revision 9
# baseline (speedup 1.0000x reference)
"""DeepSet GNN kernel, slot-padded streaming design (design C).

Key idea: pad each segment's neighbor rows into fixed 40-row *slots* so
segment_sum becomes (a) a fixed-stride windowed reduction over h plus (b) a
tiny per-window slot->segment merge matmul. This removes the per-128-row
one-hot builds (DVE) and per-block LDWEIGHTS (PE) of the one-hot design:

  - X^T is packed two-halves-high [128, NCOL/2] (full DMA port width):
    partitions 0:64 = features of "half A" slots, 64:128 = "half B" slots.
  - mm1: ONE full-array matmul per 512-col chunk with the resident
    block-diagonal stationary [[w1,0],[0,w1]] -> h^T [128, 512] in PSUM
    (partitions 0:64 = h of A-rows, 64:128 = h of B-rows).
  - ACT relu (+phi_b1 per-partition bias) -> whole-window hs in SBUF fp16.
  - One DVE windowed tensor_reduce per window over each 40-col slot
    -> slot sums [128, 64].
  - PE transpose -> [64 slots, 128]; two merge matmuls with a DVE-built
    slot->segment one-hot (ONE tensor_scalar per window)
    -> merged [SEG_W, 64] segment sums.
  - rho MLP per window; counts term via a rank-1 matmul with host counts.
"""

import os
import sys

sys.path.insert(0, "/opt/trn_rl_repo")

import numpy as np

last_results = None

def _ensure_axon_profile_hook():
    """The RL container's antenv stub lacks axon_hooks; synthesize it so a
    traced run (trace=True or BASS_TRACE=1) can capture NTFF profiles
    instead of crashing on the missing import. No-op when already present."""
    try:
        import antenv
        import importlib
        try:
            importlib.import_module("antenv.axon_hooks")
            return
        except ImportError:
            pass
        import types
        import trn_agent_boot.trn_boot as tb

        hook = tb._ntff_profile_via_ctypes("/opt/axon/libaxon_pjrt.so")
        mod = types.ModuleType("antenv.axon_hooks")
        mod._hook = hook
        mod.get_axon_ntff_profile_hook = lambda: mod._hook
        mod.set_axon_ntff_profile_hook = lambda h: setattr(mod, "_hook", h)
        sys.modules["antenv.axon_hooks"] = mod
        antenv.axon_hooks = mod
    except Exception:
        pass


N_AGENTS = 50000
N_CORES = 8
SEG_PC = N_AGENTS // N_CORES  # 6250 segments per core
P_SLOT = 40  # rows per slot
SLOT_W = 128  # slots per window (64 per half)
HALF_SLOTS = 64
WCOL = HALF_SLOTS * P_SLOT  # 2560 columns per window tile
CHUNK = 512  # cols per matmul chunk (one PSUM bank)
CHUNKS_PER_WIN = WCOL // CHUNK  # 5


def _build_program(SEG_W, NW):
    from concourse import bacc, mybir
    import concourse.tile as tile

    FP16 = mybir.dt.float16
    F32 = mybir.dt.float32
    Relu = mybir.ActivationFunctionType.Relu
    Ident = mybir.ActivationFunctionType.Identity
    Copy = mybir.ActivationFunctionType.Copy
    AX = mybir.AxisListType.X
    ADD = mybir.AluOpType.add
    ISEQ = mybir.AluOpType.is_equal

    NCOL2 = NW * WCOL
    OUTW = NW * SEG_W
    CPS = CHUNK // P_SLOT  # 12.8 -> NOT integral; hs is sliced by cols instead

    nc = bacc.Bacc("TRN2", target_bir_lowering=False, debug=False)
    xta = nc.dram_tensor("xta", [128, NCOL2], FP16, kind="ExternalInput").ap()
    w1bd = nc.dram_tensor("w1bd", [128, 128], FP16, kind="ExternalInput").ap()
    b1d = nc.dram_tensor("b1d", [128, 1], F32, kind="ExternalInput").ap()
    waa = nc.dram_tensor("waa", [64, 64], FP16, kind="ExternalInput").ap()
    vb2 = nc.dram_tensor("vb2", [1, 64], FP16, kind="ExternalInput").ap()
    wba = nc.dram_tensor("wba", [64, 2], FP16, kind="ExternalInput").ap()
    rb1 = nc.dram_tensor("rb1", [64, 1], F32, kind="ExternalInput").ap()
    rb2 = nc.dram_tensor("rb2", [2, 1], F32, kind="ExternalInput").ap()
    iotas = nc.dram_tensor("iotas", [SLOT_W, SEG_W], FP16, kind="ExternalInput").ap()
    sos = nc.dram_tensor("sos", [SLOT_W, NW], F32, kind="ExternalInput").ap()
    cntr = nc.dram_tensor("cntr", [1, OUTW], FP16, kind="ExternalInput").ap()
    idenf = nc.dram_tensor("idenf", [128, 128], F32, kind="ExternalInput").ap()
    idenh = nc.dram_tensor("idenh", [128, 128], FP16, kind="ExternalInput").ap()
    out = nc.dram_tensor("out", [2, OUTW], F32, kind="ExternalOutput").ap()

    with tile.TileContext(nc) as tc:
        with (
            tc.tile_pool(name="const", bufs=1) as cpool,
            tc.tile_pool(name="x", bufs=4) as xpool,
            tc.tile_pool(name="h", bufs=3) as hpool,
            tc.tile_pool(name="ss", bufs=8) as spool,
            tc.tile_pool(name="mg", bufs=14) as mpool,
            tc.tile_pool(name="rho", bufs=4) as rpool,
            tc.tile_pool(name="psh", bufs=4, space="PSUM") as psh,
            tc.tile_pool(name="pst1", bufs=1, space="PSUM") as pst1,
            tc.tile_pool(name="pss", bufs=1, space="PSUM") as pss,
            tc.tile_pool(name="pse", bufs=2, space="PSUM") as pse,
        ):
            w1bd_t = cpool.tile([128, 128], FP16)
            nc.sync.dma_start(w1bd_t[:], w1bd[:, :])
            b1d_t = cpool.tile([128, 1], F32)
            nc.sync.dma_start(b1d_t[:], b1d[:, :])
            waa_t = cpool.tile([64, 64], FP16)
            nc.sync.dma_start(waa_t[:], waa[:, :])
            vb2_t = cpool.tile([1, 64], FP16)
            nc.sync.dma_start(vb2_t[:], vb2[:, :])
            wba_t = cpool.tile([64, 2], FP16)
            nc.sync.dma_start(wba_t[:], wba[:, :])
            rb1_t = cpool.tile([64, 1], F32)
            nc.sync.dma_start(rb1_t[:], rb1[:, :])
            rb2_t = cpool.tile([2, 1], F32)
            nc.sync.dma_start(rb2_t[:], rb2[:, :])
            iotas_t = cpool.tile([SLOT_W, SEG_W], FP16)
            nc.sync.dma_start(iotas_t[:], iotas[:, :])
            sosa_t = cpool.tile([HALF_SLOTS, NW], F32)
            nc.sync.dma_start(sosa_t[:], sos[0:HALF_SLOTS, :])
            sosb_t = cpool.tile([HALF_SLOTS, NW], F32)
            nc.sync.dma_start(sosb_t[:], sos[HALF_SLOTS:SLOT_W, :])
            cntr_t = cpool.tile([1, OUTW], FP16)
            nc.sync.dma_start(cntr_t[:], cntr[:, :])
            idenf_t = cpool.tile([128, 128], F32)
            nc.sync.dma_start(idenf_t[:], idenf[:, :])
            idenh_t = cpool.tile([128, 128], FP16)
            nc.sync.dma_start(idenh_t[:], idenh[:, :])
            out_t = cpool.tile([2, OUTW], F32)

            pend = {}

            def front(w):
                xt = xpool.tile([128, WCOL], FP16)
                nc.sync.dma_start(xt[:], xta[:, WCOL * w : WCOL * (w + 1)])
                hs = hpool.tile([128, HALF_SLOTS, P_SLOT], FP16)
                for k in range(CHUNKS_PER_WIN):
                    hp = psh.tile([128, CHUNK], F32)
                    nc.tensor.matmul(
                        hp[:, :],
                        lhsT=w1bd_t[:],
                        rhs=xt[:, CHUNK * k : CHUNK * (k + 1)],
                        start=True,
                        stop=True,
                    )
                    hsf = hs[:, :, :].rearrange("p a b -> p (a b)")
                    nc.scalar.activation(
                        hsf[:, CHUNK * k : CHUNK * (k + 1)], hp[:, :], Relu,
                        bias=b1d_t[:],
                    )
                t1 = hpool.tile([128, HALF_SLOTS, 20], FP16, tag="t1")
                nc.vector.tensor_tensor(
                    out=t1[:, :, :], in0=hs[:, :, 0:20], in1=hs[:, :, 20:40], op=ADD
                )
                t2 = hpool.tile([128, HALF_SLOTS, 10], FP16, tag="t2")
                nc.vector.tensor_tensor(
                    out=t2[:, :, :], in0=t1[:, :, 0:10], in1=t1[:, :, 10:20], op=ADD
                )
                ssum = spool.tile([128, HALF_SLOTS], F32)
                nc.vector.tensor_reduce(ssum[:], t2[:, :, :], axis=AX, op=ADD)
                # merge one-hots on the otherwise-idle GpSimd engine
                mga = mpool.tile([HALF_SLOTS, SEG_W], FP16)
                nc.vector.tensor_scalar(
                    out=mga[:], in0=iotas_t[0:HALF_SLOTS, :],
                    scalar1=sosa_t[:, w : w + 1],
                    scalar2=None, op0=ISEQ,
                )
                mgb = mpool.tile([HALF_SLOTS, SEG_W], FP16)
                nc.vector.tensor_scalar(
                    out=mgb[:], in0=iotas_t[0:HALF_SLOTS, :],
                    scalar1=sosb_t[:, w : w + 1],
                    scalar2=None, op0=ISEQ,
                )
                pend[w] = (ssum, mga, mgb)

            def back_merge(w):
                ssum, mga, mgb = pend.pop(w)
                st1 = pst1.tile([HALF_SLOTS, 128], F32)
                nc.tensor.transpose(st1[:], ssum[:], idenf_t[:])
                stc = spool.tile([HALF_SLOTS, 128], FP16)
                nc.vector.tensor_copy(stc[:], st1[:])
                merged = pss.tile([64, SEG_W], F32)
                nc.tensor.matmul(merged[:], lhsT=stc[:, 0:64], rhs=mga[:], start=True, stop=False)
                nc.tensor.matmul(merged[:], lhsT=stc[:, 64:128], rhs=mgb[:], start=False, stop=True)
                return merged

            def back_pair(w):
                m0 = back_merge(w)
                m1 = back_merge(w + 1)
                st2_sb = rpool.tile([64, 2 * SEG_W], FP16)
                nc.vector.tensor_copy(st2_sb[:, 0:SEG_W], m0[:])
                nc.vector.tensor_copy(st2_sb[:, SEG_W : 2 * SEG_W], m1[:])
                r_ps = pse.tile([64, 2 * SEG_W], F32, tag="epi")
                nc.tensor.matmul(r_ps[:], lhsT=waa_t[:], rhs=st2_sb[:], start=True, stop=False)
                nc.tensor.matmul(
                    r_ps[:], lhsT=vb2_t[:],
                    rhs=cntr_t[:, SEG_W * w : SEG_W * (w + 2)],
                    start=False, stop=True,
                )
                r_sb = rpool.tile([64, 2 * SEG_W], FP16)
                nc.scalar.activation(r_sb[:], r_ps[:], Relu, bias=rb1_t[:])
                o_ps = pse.tile([2, 2 * SEG_W], F32, tag="epi")
                nc.tensor.matmul(o_ps[:], lhsT=wba_t[:], rhs=r_sb[:], start=True, stop=True)
                nc.scalar.activation(
                    out_t[:, SEG_W * w : SEG_W * (w + 2)], o_ps[:], Ident, bias=rb2_t[:]
                )

            DELAY = 4
            assert NW % 2 == 0
            for w in range(NW):
                front(w)
                if w >= DELAY + 1 and (w - DELAY) % 2 == 1:
                    back_pair(w - DELAY - 1)
            for w in range(NW - DELAY, NW, 2):
                back_pair(w)
            nc.sync.dma_start(out[:, :], out_t[:])
    nc.compile()
    return nc


def _host_prep(neighbors, phi_w1, phi_b1, phi_w2, phi_b2,
               rho_w1, rho_b1, rho_w2, rho_b2, segment_ids):
    ids = np.asarray(segment_ids)
    X = np.asarray(neighbors)
    r0 = np.searchsorted(ids, np.arange(N_AGENTS + 1))
    d = np.diff(r0)  # rows per segment
    kslots = -(-d // P_SLOT)  # ceil; 0 for empty segments

    SEG_W = None
    for cand in (108, 104, 100, 96, 88, 80):
        ok = True
        for c in range(N_CORES):
            ks = kslots[SEG_PC * c : SEG_PC * (c + 1)]
            nw = -(-SEG_PC // cand)
            pad = np.zeros(nw * cand, np.int64)
            pad[: SEG_PC] = ks
            if pad.reshape(nw, cand).sum(1).max() > SLOT_W:
                ok = False
                break
        if ok:
            SEG_W = cand
            break
    assert SEG_W is not None, "no SEG_W candidate fits the slot budget"
    NW = -(-SEG_PC // SEG_W)
    OUTW = NW * SEG_W

    XT = np.ascontiguousarray(X.T).astype(np.float16)  # [64, N]

    w1bd = np.zeros((128, 128), np.float32)
    w1bd[0:64, 0:64] = phi_w1
    w1bd[64:128, 64:128] = phi_w1

    consts = dict(
        w1bd=w1bd.astype(np.float16),
        b1d=np.concatenate([phi_b1, phi_b1], 0).reshape(128, 1).astype(np.float32),
        waa=(phi_w2 @ rho_w1).astype(np.float16),
        vb2=np.asarray(phi_b2 @ rho_w1).reshape(1, 64).astype(np.float16),
        wba=np.asarray(rho_w2).astype(np.float16),
        rb1=np.asarray(rho_b1).reshape(64, 1).astype(np.float32),
        rb2=np.asarray(rho_b2).reshape(2, 1).astype(np.float32),
        iotas=np.tile(np.arange(SEG_W, dtype=np.float32), (SLOT_W, 1)).astype(np.float16),
        idenf=np.eye(128, dtype=np.float32),
        idenh=np.eye(128, dtype=np.float16),
    )

    in_maps = []
    for c in range(N_CORES):
        sos = np.full((SLOT_W, NW), -1.0, np.float32)
        cnt = np.zeros((SEG_W, NW), np.float32)
        colmap = np.full((NW, 2, WCOL), -1, np.int64)
        for w in range(NW):
            lo = SEG_PC * c + SEG_W * w
            hi = min(lo + SEG_W, SEG_PC * (c + 1))
            cnt[0 : hi - lo, w] = d[lo:hi]
            si = 0
            for s in range(lo, hi):
                for j in range(kslots[s]):
                    ln = min(P_SLOT, d[s] - P_SLOT * j)
                    half, jj = divmod(si, HALF_SLOTS)
                    c0 = jj * P_SLOT
                    colmap[w, half, c0 : c0 + ln] = r0[s] + P_SLOT * j + np.arange(ln)
                    sos[si, w] = s - lo
                    si += 1
        xta = np.zeros((128, NW * WCOL), np.float16)
        for half in range(2):
            cm = colmap[:, half, :].reshape(-1)
            g = XT[:, np.clip(cm, 0, None)]
            g[:, cm < 0] = 0
            xta[64 * half : 64 * half + 64, :] = g
        in_maps.append(dict(
            xta=xta,
            sos=sos,
            cntr=np.ascontiguousarray(cnt.T.reshape(1, OUTW)).astype(np.float16),
            **consts,
        ))
    return SEG_W, NW, in_maps


def kernel(**inputs):
    global last_results
    np_inputs = {k: np.asarray(v) for k, v in inputs.items()}
    SEG_W, NW, in_maps = _host_prep(**np_inputs)
    nc = _build_program(SEG_W, NW)

    _ensure_axon_profile_hook()
    from concourse.bass_utils import run_bass_kernel_spmd

    trace = bool(os.environ.get("KERNEL_TRACE"))
    res = run_bass_kernel_spmd(nc, in_maps, list(range(N_CORES)), trace=trace)
    if trace:
        last_results = res
    cols = []
    for c in range(N_CORES):
        cols.append(res.results[c]["out"][:, :SEG_PC])
    out_t = np.concatenate(cols, 1)  # [2, 50000]
    return np.ascontiguousarray(out_t.T).astype(np.float32)


# revision 10
# speedup vs baseline: 1.4368x; 1.4368x over previous
"""DeepSet GNN kernel, slot-padded streaming design (design C).

Key idea: pad each segment's neighbor rows into fixed 40-row *slots* so
segment_sum becomes (a) a fixed-stride windowed reduction over h plus (b) a
tiny per-window slot->segment merge matmul. This removes the per-128-row
one-hot builds (DVE) and per-block LDWEIGHTS (PE) of the one-hot design:

  - X^T is packed two-halves-high [128, NCOL/2] (full DMA port width):
    partitions 0:64 = features of "half A" slots, 64:128 = "half B" slots.
  - mm1: ONE full-array matmul per 512-col chunk with the resident
    block-diagonal stationary [[w1,0],[0,w1]] -> h^T [128, 512] in PSUM
    (partitions 0:64 = h of A-rows, 64:128 = h of B-rows).
  - ACT relu (+phi_b1 per-partition bias) -> whole-window hs in SBUF fp16.
  - One DVE windowed tensor_reduce per window over each 40-col slot
    -> slot sums [128, 64].
  - PE transpose -> [64 slots, 128]; two merge matmuls with a DVE-built
    slot->segment one-hot (ONE tensor_scalar per window)
    -> merged [SEG_W, 64] segment sums.
  - rho MLP per window; counts term via a rank-1 matmul with host counts.
"""

import os
import sys

sys.path.insert(0, "/opt/trn_rl_repo")

import numpy as np

last_results = None

def _ensure_axon_profile_hook():
    """The RL container's antenv stub lacks axon_hooks; synthesize it so a
    traced run (trace=True or BASS_TRACE=1) can capture NTFF profiles
    instead of crashing on the missing import. No-op when already present."""
    try:
        import antenv
        import importlib
        try:
            importlib.import_module("antenv.axon_hooks")
            return
        except ImportError:
            pass
        import types
        import trn_agent_boot.trn_boot as tb

        hook = tb._ntff_profile_via_ctypes("/opt/axon/libaxon_pjrt.so")
        mod = types.ModuleType("antenv.axon_hooks")
        mod._hook = hook
        mod.get_axon_ntff_profile_hook = lambda: mod._hook
        mod.set_axon_ntff_profile_hook = lambda h: setattr(mod, "_hook", h)
        sys.modules["antenv.axon_hooks"] = mod
        antenv.axon_hooks = mod
    except Exception:
        pass


N_AGENTS = 50000
N_CORES = 8
SEG_PC = N_AGENTS // N_CORES  # 6250 segments per core
P_SLOT = 40  # rows per slot
SLOT_W = 128  # slots per window (64 per half)
HALF_SLOTS = 64
WCOL = HALF_SLOTS * P_SLOT  # 2560 columns per window tile
CHUNK = 512  # cols per matmul chunk (one PSUM bank)
CHUNKS_PER_WIN = WCOL // CHUNK  # 5


def _build_program(SEG_W, NW):
    from concourse import bacc, mybir
    import concourse.tile as tile

    FP16 = mybir.dt.float16
    F32 = mybir.dt.float32
    Relu = mybir.ActivationFunctionType.Relu
    Ident = mybir.ActivationFunctionType.Identity
    Copy = mybir.ActivationFunctionType.Copy
    AX = mybir.AxisListType.X
    ADD = mybir.AluOpType.add
    ISEQ = mybir.AluOpType.is_equal

    NCOL2 = NW * WCOL
    OUTW = NW * SEG_W
    CPS = CHUNK // P_SLOT  # 12.8 -> NOT integral; hs is sliced by cols instead

    nc = bacc.Bacc("TRN2", target_bir_lowering=False, debug=False)
    xta = nc.dram_tensor("xta", [128, NCOL2], FP16, kind="ExternalInput").ap()
    w1bd = nc.dram_tensor("w1bd", [128, 128], FP16, kind="ExternalInput").ap()
    b1d = nc.dram_tensor("b1d", [128, 1], F32, kind="ExternalInput").ap()
    waa = nc.dram_tensor("waa", [64, 64], FP16, kind="ExternalInput").ap()
    vb2 = nc.dram_tensor("vb2", [1, 64], FP16, kind="ExternalInput").ap()
    wba = nc.dram_tensor("wba", [64, 2], FP16, kind="ExternalInput").ap()
    rb1 = nc.dram_tensor("rb1", [64, 1], F32, kind="ExternalInput").ap()
    rb2 = nc.dram_tensor("rb2", [2, 1], F32, kind="ExternalInput").ap()
    iotas = nc.dram_tensor("iotas", [SLOT_W, SEG_W], FP16, kind="ExternalInput").ap()
    sos = nc.dram_tensor("sos", [SLOT_W, NW], F32, kind="ExternalInput").ap()
    cntr = nc.dram_tensor("cntr", [1, OUTW], FP16, kind="ExternalInput").ap()
    idenf = nc.dram_tensor("idenf", [128, 128], F32, kind="ExternalInput").ap()
    idenh = nc.dram_tensor("idenh", [128, 128], FP16, kind="ExternalInput").ap()
    out = nc.dram_tensor("out", [2, OUTW], F32, kind="ExternalOutput").ap()

    with tile.TileContext(nc) as tc:
        with (
            tc.tile_pool(name="const", bufs=1) as cpool,
            tc.tile_pool(name="x", bufs=4) as xpool,
            tc.tile_pool(name="h", bufs=3) as hpool,
            tc.tile_pool(name="ss", bufs=5) as spool,
            tc.tile_pool(name="mg", bufs=8) as mpool,
            tc.tile_pool(name="rho", bufs=3) as rpool,
            tc.tile_pool(name="psh", bufs=4, space="PSUM") as psh,
            tc.tile_pool(name="pst1", bufs=1, space="PSUM") as pst1,
            tc.tile_pool(name="pss", bufs=1, space="PSUM") as pss,
            tc.tile_pool(name="pse", bufs=2, space="PSUM") as pse,
        ):
            w1bd_t = cpool.tile([128, 128], FP16)
            nc.sync.dma_start(w1bd_t[:], w1bd[:, :])
            b1d_t = cpool.tile([128, 1], F32)
            nc.sync.dma_start(b1d_t[:], b1d[:, :])
            waa_t = cpool.tile([64, 64], FP16)
            nc.sync.dma_start(waa_t[:], waa[:, :])
            vb2_t = cpool.tile([1, 64], FP16)
            nc.sync.dma_start(vb2_t[:], vb2[:, :])
            wba_t = cpool.tile([64, 2], FP16)
            nc.sync.dma_start(wba_t[:], wba[:, :])
            rb1_t = cpool.tile([64, 1], F32)
            nc.sync.dma_start(rb1_t[:], rb1[:, :])
            rb2_t = cpool.tile([2, 1], F32)
            nc.sync.dma_start(rb2_t[:], rb2[:, :])
            iotas_t = cpool.tile([SLOT_W, SEG_W], FP16)
            nc.sync.dma_start(iotas_t[:], iotas[:, :])
            sosa_t = cpool.tile([HALF_SLOTS, NW], F32)
            nc.sync.dma_start(sosa_t[:], sos[0:HALF_SLOTS, :])
            sosb_t = cpool.tile([HALF_SLOTS, NW], F32)
            nc.sync.dma_start(sosb_t[:], sos[HALF_SLOTS:SLOT_W, :])
            cntr_t = cpool.tile([1, OUTW], FP16)
            nc.sync.dma_start(cntr_t[:], cntr[:, :])
            idenf_t = cpool.tile([128, 128], F32)
            nc.sync.dma_start(idenf_t[:], idenf[:, :])
            idenh_t = cpool.tile([128, 128], FP16)
            nc.sync.dma_start(idenh_t[:], idenh[:, :])
            out_t = cpool.tile([2, OUTW], F32)

            pend = {}

            def front(w):
                xt = xpool.tile([128, WCOL], FP16)
                nc.sync.dma_start(xt[:], xta[:, WCOL * w : WCOL * (w + 1)])
                hs = hpool.tile([128, HALF_SLOTS, P_SLOT], FP16)
                for k in range(CHUNKS_PER_WIN):
                    hp = psh.tile([128, CHUNK], F32)
                    nc.tensor.matmul(
                        hp[:, :],
                        lhsT=w1bd_t[:],
                        rhs=xt[:, CHUNK * k : CHUNK * (k + 1)],
                        start=True,
                        stop=True,
                    )
                    hsf = hs[:, :, :].rearrange("p a b -> p (a b)")
                    nc.scalar.activation(
                        hsf[:, CHUNK * k : CHUNK * (k + 1)], hp[:, :], Relu,
                        bias=b1d_t[:],
                    )
                t1 = hpool.tile([128, HALF_SLOTS, 20], FP16, tag="t1")
                nc.vector.tensor_tensor(
                    out=t1[:, :, :], in0=hs[:, :, 0:20], in1=hs[:, :, 20:40], op=ADD
                )
                t2 = hpool.tile([128, HALF_SLOTS, 10], FP16, tag="t2")
                nc.vector.tensor_tensor(
                    out=t2[:, :, :], in0=t1[:, :, 0:10], in1=t1[:, :, 10:20], op=ADD
                )
                ssum = spool.tile([128, HALF_SLOTS], F32)
                nc.vector.tensor_reduce(ssum[:], t2[:, :, :], axis=AX, op=ADD)
                # merge one-hots on the otherwise-idle GpSimd engine
                mga = mpool.tile([HALF_SLOTS, SEG_W], FP16)
                nc.vector.tensor_scalar(
                    out=mga[:], in0=iotas_t[0:HALF_SLOTS, :],
                    scalar1=sosa_t[:, w : w + 1],
                    scalar2=None, op0=ISEQ,
                )
                mgb = mpool.tile([HALF_SLOTS, SEG_W], FP16)
                nc.vector.tensor_scalar(
                    out=mgb[:], in0=iotas_t[0:HALF_SLOTS, :],
                    scalar1=sosb_t[:, w : w + 1],
                    scalar2=None, op0=ISEQ,
                )
                pend[w] = (ssum, mga, mgb)

            def back_merge(w):
                ssum, mga, mgb = pend.pop(w)
                st1 = pst1.tile([HALF_SLOTS, 128], F32)
                nc.tensor.transpose(st1[:], ssum[:], idenf_t[:])
                stc = spool.tile([HALF_SLOTS, 128], FP16)
                nc.vector.tensor_copy(stc[:], st1[:])
                merged = pss.tile([64, SEG_W], F32)
                nc.tensor.matmul(merged[:], lhsT=stc[:, 0:64], rhs=mga[:], start=True, stop=False)
                nc.tensor.matmul(merged[:], lhsT=stc[:, 64:128], rhs=mgb[:], start=False, stop=True)
                return merged

            def back_pair(w):
                m0 = back_merge(w)
                m1 = back_merge(w + 1)
                st2_sb = rpool.tile([64, 2 * SEG_W], FP16)
                nc.vector.tensor_copy(st2_sb[:, 0:SEG_W], m0[:])
                nc.vector.tensor_copy(st2_sb[:, SEG_W : 2 * SEG_W], m1[:])
                r_ps = pse.tile([64, 2 * SEG_W], F32, tag="epi")
                nc.tensor.matmul(r_ps[:], lhsT=waa_t[:], rhs=st2_sb[:], start=True, stop=False)
                nc.tensor.matmul(
                    r_ps[:], lhsT=vb2_t[:],
                    rhs=cntr_t[:, SEG_W * w : SEG_W * (w + 2)],
                    start=False, stop=True,
                )
                r_sb = rpool.tile([64, 2 * SEG_W], FP16)
                nc.scalar.activation(r_sb[:], r_ps[:], Relu, bias=rb1_t[:])
                o_ps = pse.tile([2, 2 * SEG_W], F32, tag="epi")
                nc.tensor.matmul(o_ps[:], lhsT=wba_t[:], rhs=r_sb[:], start=True, stop=True)
                nc.scalar.activation(
                    out_t[:, SEG_W * w : SEG_W * (w + 2)], o_ps[:], Ident, bias=rb2_t[:]
                )

            DELAY = 4
            assert NW % 2 == 0
            for w in range(NW):
                front(w)
                if w >= DELAY + 1 and (w - DELAY) % 2 == 1:
                    back_pair(w - DELAY - 1)
            for w in range(NW - DELAY, NW, 2):
                back_pair(w)
            nc.sync.dma_start(out[:, :], out_t[:])
    nc.compile()
    return nc


def _host_prep(neighbors, phi_w1, phi_b1, phi_w2, phi_b2,
               rho_w1, rho_b1, rho_w2, rho_b2, segment_ids):
    ids = np.asarray(segment_ids)
    X = np.asarray(neighbors)
    r0 = np.searchsorted(ids, np.arange(N_AGENTS + 1))
    d = np.diff(r0)  # rows per segment
    kslots = -(-d // P_SLOT)  # ceil; 0 for empty segments

    SEG_W = None
    for cand in (108, 104, 100, 96, 88, 80):
        ok = True
        for c in range(N_CORES):
            ks = kslots[SEG_PC * c : SEG_PC * (c + 1)]
            nw = -(-SEG_PC // cand)
            pad = np.zeros(nw * cand, np.int64)
            pad[: SEG_PC] = ks
            if pad.reshape(nw, cand).sum(1).max() > SLOT_W:
                ok = False
                break
        if ok:
            SEG_W = cand
            break
    assert SEG_W is not None, "no SEG_W candidate fits the slot budget"
    NW = -(-SEG_PC // SEG_W)
    OUTW = NW * SEG_W

    XT = np.ascontiguousarray(X.T).astype(np.float16)  # [64, N]

    w1bd = np.zeros((128, 128), np.float32)
    w1bd[0:64, 0:64] = phi_w1
    w1bd[64:128, 64:128] = phi_w1

    consts = dict(
        w1bd=w1bd.astype(np.float16),
        b1d=np.concatenate([phi_b1, phi_b1], 0).reshape(128, 1).astype(np.float32),
        waa=(phi_w2 @ rho_w1).astype(np.float16),
        vb2=np.asarray(phi_b2 @ rho_w1).reshape(1, 64).astype(np.float16),
        wba=np.asarray(rho_w2).astype(np.float16),
        rb1=np.asarray(rho_b1).reshape(64, 1).astype(np.float32),
        rb2=np.asarray(rho_b2).reshape(2, 1).astype(np.float32),
        iotas=np.tile(np.arange(SEG_W, dtype=np.float32), (SLOT_W, 1)).astype(np.float16),
        idenf=np.eye(128, dtype=np.float32),
        idenh=np.eye(128, dtype=np.float16),
    )

    in_maps = []
    for c in range(N_CORES):
        sos = np.full((SLOT_W, NW), -1.0, np.float32)
        cnt = np.zeros((SEG_W, NW), np.float32)
        colmap = np.full((NW, 2, WCOL), -1, np.int64)
        for w in range(NW):
            lo = SEG_PC * c + SEG_W * w
            hi = min(lo + SEG_W, SEG_PC * (c + 1))
            cnt[0 : hi - lo, w] = d[lo:hi]
            si = 0
            for s in range(lo, hi):
                for j in range(kslots[s]):
                    ln = min(P_SLOT, d[s] - P_SLOT * j)
                    half, jj = divmod(si, HALF_SLOTS)
                    c0 = jj * P_SLOT
                    colmap[w, half, c0 : c0 + ln] = r0[s] + P_SLOT * j + np.arange(ln)
                    sos[si, w] = s - lo
                    si += 1
        xta = np.zeros((128, NW * WCOL), np.float16)
        for half in range(2):
            cm = colmap[:, half, :].reshape(-1)
            g = XT[:, np.clip(cm, 0, None)]
            g[:, cm < 0] = 0
            xta[64 * half : 64 * half + 64, :] = g
        in_maps.append(dict(
            xta=xta,
            sos=sos,
            cntr=np.ascontiguousarray(cnt.T.reshape(1, OUTW)).astype(np.float16),
            **consts,
        ))
    return SEG_W, NW, in_maps


def kernel(**inputs):
    global last_results
    np_inputs = {k: np.asarray(v) for k, v in inputs.items()}
    SEG_W, NW, in_maps = _host_prep(**np_inputs)
    nc = _build_program(SEG_W, NW)

    _ensure_axon_profile_hook()
    from concourse.bass_utils import run_bass_kernel_spmd

    trace = bool(os.environ.get("KERNEL_TRACE"))
    res = run_bass_kernel_spmd(nc, in_maps, list(range(N_CORES)), trace=trace)
    if trace:
        last_results = res
    cols = []
    for c in range(N_CORES):
        cols.append(res.results[c]["out"][:, :SEG_PC])
    out_t = np.concatenate(cols, 1)  # [2, 50000]
    return np.ascontiguousarray(out_t.T).astype(np.float32)
